# revision 27
# baseline (speedup 1.0000x reference)
"""Quantized matmul (uint4 groupwise dequant) on 8 Trainium2 NeuronCores.

Computes out = a_f32 @ W where W[k, n] = (q[k, n] - zeros[k//128, n]) * scales[k//128, n].

Sharding: tensor-parallel along N (output features). Each of the 8 cores gets
N_LOCAL = 512 columns of q/scales/zeros and the full `a` (replicated). Each
core dequantizes its W slice to fp16 once into SBUF, then runs a dense
fp16 matmul with fp32 PSUM accumulation.

Device kernel layout choices (all host-side prep is pure layout/sharding):
 - `a` is fed pre-transposed and tiled as aT[m_out, k_in, k_out*128 + m_in]
   so each [128, 4096] SBUF tile is one contiguous 1 MiB DMA and slices
   [:, k*128:(k+1)*128] are matmul lhsT tiles (K on partitions).
 - q values are 0..15, so the int32 container is narrowed to int8 on the
   host (lossless) to quarter its DMA cost; the DVE subtract consumes the
   int8 operand directly (int8 - fp16 -> fp16 in one op).
 - scales/zeros come in as [32, 512] slices; both are broadcast across the
   128 partitions on-device with chunked stride-0 DRAM->SBUF DMAs.

Schedule: the PE warms up with 6 dummy matmuls (the HAM clock gate needs
~3.4us of activity to reach 8/8 = 2.4 GHz), then runs a single
availability-ordered wavefront covering m-tiles 0..14: aT quarters for
m0/m1 give the PE work in the first microseconds, and the extension to 15
m-tiles (PSUM banks recycled with explicit ordering keys) gives the
in-order PE stream enough backlog to absorb the W-dequant trickle without
going idle. Remaining m-tiles run m-outer/k-inner with inline epilogues;
the last epilogue is split across ACT+DVE and two DMA queues.
"""

import numpy as np

M, K, N = 4096, 4096, 4096
G = 128          # quant group size
P = 128          # partitions
NCORES = 8
NL = N // NCORES          # 512 output columns per core
KT = K // P               # 32 k tiles (== quant groups)
MT = M // P               # 32 m tiles
NQM = 2                   # m-tiles loaded as quarters (m0, m1)
NFULL = 6                 # early full m-tiles (m2..m7)
WAVE = 15                 # m-tiles in the availability-sorted wavefront
AQ = 4                    # quarters per quartered m-tile
WARMUP = 7

_CACHE = {}


def _build_nc():
    import concourse.bacc as bacc
    import concourse.mybir as mybir
    import concourse.tile as tile
    from concourse.bass import ts

    f16 = mybir.dt.float16
    f32 = mybir.dt.float32
    i8 = mybir.dt.int8

    nc = bacc.Bacc("TRN2", target_bir_lowering=False, debug=False)

    aT = nc.dram_tensor("aT", [MT, P, K], f16, kind="ExternalInput").ap()
    q = nc.dram_tensor("q", [KT, P, NL], i8, kind="ExternalInput").ap()
    zsm = nc.dram_tensor("zsm", [1, KT * NL], f16, kind="ExternalInput").ap()
    ssm = nc.dram_tensor("ssm", [1, KT * NL], f16, kind="ExternalInput").ap()
    out = nc.dram_tensor("out", [MT, P, NL], f32, kind="ExternalOutput").ap()

    with tile.TileContext(nc) as tc:
        CHUNKS = [1, 1, 1, 1, 2, 2, 4, 4, 4, 4, 4, 4]
        assert sum(CHUNKS) == KT

        # Availability model (us, relative to DMA boot) used to order the
        # wavefront: cumulative emitted HBM bytes over ~0.358 MB/us plus
        # the serial DVE dequant pipeline.
        RATE = 0.358
        DVE_G = 0.95

        avail_w = [0.0] * KT
        avail_aq = {}             # (mi, quarter) -> ready time, mi < NQM
        avail_at = [0.0] * WAVE   # full-tile ready times

        with (
            tc.tile_pool(name="w", bufs=KT) as wpool,
            tc.tile_pool(name="zsb", bufs=3) as zsbpool,
            tc.tile_pool(name="qraw", bufs=6) as qpool,
            tc.tile_pool(name="deq", bufs=4) as dqpool,
            tc.tile_pool(name="a0", bufs=NQM * AQ) as a0pool,
            tc.tile_pool(name="atb", bufs=NFULL) as bpool,
            tc.tile_pool(name="at", bufs=7) as apool,
            tc.tile_pool(name="ot", bufs=2) as opool,
            tc.tile_pool(name="ps", bufs=8, space="PSUM") as pspool,
        ):
            cum_mb = 0.0
            dve_free = 0.0
            aqt = {}              # (mi, v) -> quarter tile
            ats = [None] * WAVE   # full tiles (mi >= NQM)
            w_tiles = []

            # PE warm-up: dummy matmuls pull the HAM clock gate to 8/8
            # before real operands arrive.
            warm_in = dqpool.tile([P, NL], f16, name="warm_in", tag="d")
            nc.gpsimd.memset(warm_in[:], 0.0)
            warm_ps = pspool.tile([P, NL], f32, name="warm_ps", tag="ps")
            for i in range(WARMUP):
                nc.tensor.matmul(
                    warm_ps[:],
                    warm_in[:, 0:P],
                    warm_in[:],
                    start=(i == 0),
                    stop=(i == WARMUP - 1),
                )

            def emit_quarter(mi, v):
                nonlocal cum_mb
                t = a0pool.tile([P, K // AQ], f16, name=f"aq{mi}_{v}", tag="a0")
                nc.sync.dma_start(t[:], aT[mi][:, ts(v, K // AQ)])
                cum_mb += (P * K // AQ) * 2 / 1e6
                avail_aq[(mi, v)] = cum_mb / RATE
                aqt[(mi, v)] = t

            def emit_at(mi):
                nonlocal cum_mb
                pool = bpool if mi < NQM + NFULL else apool
                t = pool.tile([P, K], f16, name=f"at_{mi}", tag="at")
                nc.sync.dma_start(t[:], aT[mi])
                cum_mb += (P * K) * 2 / 1e6
                avail_at[mi] = cum_mb / RATE
                ats[mi] = t

            def emit_chunk(j, k_base, gpc):
                # The z and s broadcasts issue from DIFFERENT sequencers
                # (Scalar / Sync) so their ~3.5us-per-chunk 128-row
                # descriptor transfers run on parallel DMA rings instead of
                # serializing; q loads go through GpSimd (SWDGE). The
                # per-group sub+mul alternates DVE / GpSimd so the dequant
                # engine chain isn't the rate limit either.
                nonlocal cum_mb, dve_free
                zbc = zsbpool.tile([P, gpc * NL], f16, name=f"zbc{j}", tag="zb")
                nc.scalar.dma_start(
                    zbc[:],
                    zsm[:, k_base * NL : (k_base + gpc) * NL].partition_broadcast(P),
                )
                sbc = zsbpool.tile([P, gpc * NL], f16, name=f"sbc{j}", tag="sb")
                nc.sync.dma_start(
                    sbc[:],
                    ssm[:, k_base * NL : (k_base + gpc) * NL].partition_broadcast(P),
                )
                cum_mb += 2 * (P * gpc * NL) * 2 / 1e6
                qt = qpool.tile([P, gpc, NL], i8, name=f"qt{j}", tag="qt")
                nc.gpsimd.dma_start(
                    qt[:],
                    q[k_base : k_base + gpc].rearrange("g p n -> p g n"),
                )
                cum_mb += (P * gpc * NL) / 1e6
                for g in range(gpc):
                    k = k_base + g
                    eng = nc.vector if g % 2 == 0 else nc.gpsimd
                    d = dqpool.tile([P, NL], f16, tag="d")
                    eng.tensor_sub(
                        out=d[:], in0=qt[:, g, :], in1=zbc[:, ts(g, NL)]
                    )
                    wt = wpool.tile([P, NL], f16, tag="w")
                    eng.tensor_mul(out=wt[:], in0=d[:], in1=sbc[:, ts(g, NL)])
                    w_tiles.append(wt)
                    dve_free = max(dve_free, cum_mb / RATE) + DVE_G / 2
                    avail_w[k] = dve_free

            # Emission order: two tiny chunks lead (shortest path to W_0),
            # quarters of m0/m1 interleaved with more small chunks, then
            # the W pipeline with early aT tiles spread between chunks,
            # then the extension tiles.
            plan = [("c", 0), ("c", 1)]
            for v in range(AQ):
                plan += [("q", 0, v), ("q", 1, v)]
                if v < 3:
                    plan.append(("c", 2 + v))
            plan.append(("c", 5))
            ai = NQM
            for j in range(6, len(CHUNKS)):
                plan += [("a", ai), ("c", j)]
                ai += 1
            while ai < WAVE:
                plan.append(("a", ai))
                ai += 1

            kbases = [0]
            for gpc in CHUNKS:
                kbases.append(kbases[-1] + gpc)
            for item in plan:
                if item[0] == "c":
                    j = item[1]
                    emit_chunk(j, kbases[j], CHUNKS[j])
                elif item[0] == "q":
                    emit_quarter(item[1], item[2])
                else:
                    emit_at(item[1])

            def lhsT(mi, k):
                if mi < NQM:
                    return aqt[(mi, k * AQ // KT)][:, ts(k % (KT // AQ), P)]
                return ats[mi][:, ts(k, P)]

            def avail_lhs(mi, k):
                if mi < NQM:
                    return avail_aq[(mi, k * AQ // KT)]
                return avail_at[mi]

            # Wavefront over m0..WAVE-1, ordered by modeled availability.
            # Keys are prefix-maxed per m (k==0 carries start=True and must
            # go first) and chained across PSUM-bank reuse: tile i of the
            # pool cycle shares a bank with tile i-8, so its matmuls must
            # be emitted after the earlier tile's accumulation finished.
            pss = [
                pspool.tile([P, NL], f32, name=f"ps0_{i}", tag="ps")
                for i in range(WAVE)
            ]
            keys = {}
            for mi in range(WAVE):
                run = 0.0
                if mi >= 7:
                    # bank shared with pss[mi-8] (warm_ps offsets by one)
                    run = keys[(mi - 8, KT - 1)] if mi >= 8 else 0.0
                if mi == 7:
                    run = 0.0  # shares with warm_ps, free after warmup
                for k in range(KT):
                    run = max(run, avail_lhs(mi, k), avail_w[k])
                    keys[(mi, k)] = run
            order = sorted(
                ((mi, k) for mi in range(WAVE) for k in range(KT)),
                key=lambda t: (keys[t], t[0], t[1]),
            )
            for mi, k in order:
                nc.tensor.matmul(
                    pss[mi][:],
                    lhsT(mi, k),
                    w_tiles[k][:],
                    start=(k == 0),
                    stop=(k == KT - 1),
                )
            # Epilogues in completion order so the scalar queue drains the
            # PSUM banks in the order the wave finishes them.
            for mi in sorted(range(WAVE), key=lambda m: keys[(m, KT - 1)]):
                ot = opool.tile([P, NL], f32)
                nc.scalar.copy(ot[:], pss[mi][:])
                nc.scalar.dma_start(out[mi], ot[:])

            # Remaining m-tiles: m-outer, k-inner, inline epilogue.
            for m in range(WAVE, MT):
                at = apool.tile([P, K], f16, name=f"at_{m}", tag="at")
                nc.sync.dma_start(at[:], aT[m])
                ps = pspool.tile([P, NL], f32, name=f"ps_{m}", tag="ps")
                for k in range(KT):
                    nc.tensor.matmul(
                        ps[:],
                        at[:, ts(k, P)],
                        w_tiles[k][:],
                        start=(k == 0),
                        stop=(k == KT - 1),
                    )
                if m < MT - 1:
                    ot = opool.tile([P, NL], f32)
                    nc.scalar.copy(ot[:], ps[:])
                    nc.scalar.dma_start(out[m], ot[:])
                else:
                    # tail: split the last epilogue across ACT+DVE and two
                    # DMA queues.
                    h = NL // 2
                    ota = opool.tile([P, h], f32)
                    otb = opool.tile([P, h], f32)
                    nc.scalar.copy(ota[:], ps[:, :h])
                    nc.vector.tensor_copy(otb[:], ps[:, h:])
                    nc.scalar.dma_start(out[m][:, :h], ota[:])
                    nc.sync.dma_start(out[m][:, h:], otb[:])

    nc.compile()
    return nc


def _shard_inputs(a, q_weight, scales, zeros):
    """Host-side shard/layout. Pure slicing, transposition and replication."""
    # aT[m_out, k_in, k_out*128 + m_in] = a[m_out*128 + m_in, k_out*128 + k_in]
    aT = np.ascontiguousarray(
        a.reshape(MT, P, KT, P).transpose(0, 3, 2, 1)
    ).reshape(MT, P, K)
    # q values are 0..15: int8 container is lossless.
    q8 = q_weight.astype(np.int8)

    in_maps = []
    for c in range(NCORES):
        sl = slice(c * NL, (c + 1) * NL)
        q_c = np.ascontiguousarray(q8[:, sl]).reshape(KT, P, NL)
        z_c = np.ascontiguousarray(zeros[:, sl]).reshape(1, KT * NL)
        s_c = np.ascontiguousarray(scales[:, sl]).reshape(1, KT * NL)
        in_maps.append({"aT": aT, "q": q_c, "zsm": z_c, "ssm": s_c})
    return in_maps


def _run(inputs, trace=False):
    from concourse import bass_utils

    if "nc" not in _CACHE:
        _CACHE["nc"] = _build_nc()
    nc = _CACHE["nc"]

    a = np.asarray(inputs["a"], dtype=np.float16)
    q_weight = np.asarray(inputs["q_weight"], dtype=np.int32)
    scales = np.asarray(inputs["scales"], dtype=np.float16)
    zeros = np.asarray(inputs["zeros"], dtype=np.float16)

    in_maps = _shard_inputs(a, q_weight, scales, zeros)
    res = bass_utils.run_bass_kernel_spmd(
        nc, in_maps, core_ids=list(range(NCORES)), trace=trace
    )

    out = np.empty((M, N), dtype=np.float32)
    for c in range(NCORES):
        out[:, c * NL : (c + 1) * NL] = res.results[c]["out"].reshape(M, NL)
    return out, res


def kernel(**inputs) -> np.ndarray:
    out, _ = _run(inputs, trace=False)
    return out
